# revision 14
# baseline (speedup 1.0000x reference)
"""Trainium2 Bass kernel for nn_BaselineAttention_36172214567310.

Reference computation (note the einsum 'bhqk,bhkd->bhkd' bug: the "attention
output" is v scaled by the column-sums of the softmax matrix):

    qkv = x @ w_qkv                       # [b, s, 3*H*D]
    q, k, v = split(qkv)                  # per head
    P = softmax(q @ k^T / sqrt(D))        # [q, k] rows sum to 1
    colsum[k] = sum_q P[q, k]
    values[k, :] = v[k, :] * colsum_h[k]
    out = values @ w_o

Sharding: 8 cores = 2 batches x 4 head-groups (4 heads each).

Per-core pipeline. The ACT exp stream is the pacing engine (~180us:
128 x [128,1024] ACTIVATE + fused-rowsum drain); every other engine is
sized to stay off its critical path even when the PE is HAM-throttled
to 1.2 GHz:
  - Q/K projection in fp8 DoubleRow (K=256 per matmul), V in bf16.
  - Scores bf16, K=64, head pair A/B row-tiled (lhsT base partitions 0/64)
    so the two heads' score matmuls run concurrently; A fills U0, B fills
    U1, in [128,1024] halves ping-ponged against the exps.
  - exp on ACT -> bf16 E tiles in SBUF + accum_out rowsum per half.
  - colsum: per chunk, acc_h += E * (1/rowsum) as ONE fused
    scalar_tensor_tensor (head A on DVE, head B on GPSIMD, bf16 in/out);
    at each pair's end a tiny ones-weight matvec does the partition
    reduction into ps_c (head A via a [ones|zeros] wide weight so the
    bank-wide has_written clear covers head B's region, B accumulated at
    base partition 64), then v *= colsum straight from PSUM.
  - out = values @ w_o: pair-0 partials precomputed into SBUF (borrowing
    the idle ps_c banks during pair-1), pair-1 + re-injection of the
    pair-0 partial via an identity matmul accumulate in the tail; copies
    alternate DVE/ACT; bf16 DMA out. Host sums the 4 group partials.

PSUM: U0, U1 ([128,1024] f32, 2 banks each) + ps_c ([128,2048] f32,
4 banks) = all 8 banks. Projection leftovers borrow idle U windows.
"""

import sys

sys.path.insert(0, "/opt/trn_rl_repo")

import numpy as np

B, S, HIDDEN = 2, 2048, 1024
NH, HD = 16, 64
N_CORES = 8
P = 128
QC = S // P          # 16 q chunks of 128

_CACHE = {}


def _build():
    if "nc" in _CACHE:
        return _CACHE["nc"]

    import concourse.mybir as mybir
    import concourse.tile as tile
    from concourse import bacc

    F32 = mybir.dt.float32
    BF16 = mybir.dt.bfloat16
    F8E4 = mybir.dt.float8e4
    EXP = mybir.ActivationFunctionType.Exp
    ADD = mybir.AluOpType.add
    MULT = mybir.AluOpType.mult
    DR = mybir.MatmulPerfMode.DoubleRow

    nc = bacc.Bacc()
    xt8_d = nc.declare_dram_parameter("xt8", [P, 8, S], F8E4, isOutput=False)
    xtb_d = nc.declare_dram_parameter("xtb", [P, 8, S], BF16, isOutput=False)
    wqk8_d = nc.declare_dram_parameter("wqk8", [P, 8, 512], F8E4, isOutput=False)
    wvb_d = nc.declare_dram_parameter("wvb", [P, 8, 256], BF16, isOutput=False)
    wo_d = nc.declare_dram_parameter("wo", [256, HIDDEN], BF16, isOutput=False)
    id_d = nc.declare_dram_parameter("ident", [P, P], BF16, isOutput=False)
    out_d = nc.declare_dram_parameter("out", [S, HIDDEN], BF16, isOutput=True)

    with tile.TileContext(nc) as tc:
        with tc.tile_pool(name="sb", bufs=1) as sb, \
             tc.tile_pool(name="ps", bufs=1, space="PSUM") as ps:

            # ---- persistent SBUF tiles ----
            qt = [sb.tile([P, S], BF16, name=f"qt{g}") for g in range(2)]
            kt = [sb.tile([P, S], BF16, name=f"kt{g}") for g in range(2)]
            vt = [sb.tile([P, S], BF16, name=f"vt{g}") for g in range(2)]
            xt8 = sb.tile([P, 8, S], F8E4, name="xt8")
            xtb = sb.tile([P, 8, S], BF16, name="xtb")
            wqk8 = sb.tile([P, 8, 512], F8E4, name="wqk8")
            wvb = sb.tile([P, 8, 256], BF16, name="wvb")
            wo_t = [sb.tile([P, HIDDEN], BF16, name=f"wo{g}") for g in range(2)]
            ident = sb.tile([P, P], BF16, name="ident")
            # E + rowsums, ping-pong per head over chunk parity
            e_t = [[sb.tile([P, S], BF16, name=f"e{h}{i}") for i in range(2)]
                   for h in range(2)]
            r_t = [[sb.tile([P, 2], F32, name=f"r{h}{i}") for i in range(2)]
                   for h in range(2)]
            rs_t = [[sb.tile([P, 1], F32, name=f"rs{h}{i}") for i in range(2)]
                    for h in range(2)]
            # per-pair colsum partials (partition p holds sum over its own
            # q rows), reduced across partitions by the ones-matvec
            acc = [sb.tile([P, S], BF16, name=f"acc{h}") for h in range(2)]
            tmp_b = sb.tile([P, S], BF16, name="tmp_b")
            ones_pad = sb.tile([P, P], BF16, name="ones_pad")
            out_sb = sb.tile([P, QC, HIDDEN], BF16, name="out_sb")
            stage = [sb.tile([P, HIDDEN], BF16, name=f"stage{i}") for i in range(2)]
            warm = sb.tile([P, 1], F32, name="warm")

            # PSUM: 2+2 banks of score halves, 4 banks for colsum / borrow
            U = [ps.tile([P, 1024], F32, name=f"U{i}") for i in range(2)]
            ps_c = ps.tile([P, S], F32, name="ps_c")

            # ---- constants + exp table preload (overlap input DMA) ----
            nc.vector.memset(warm, 0.0)
            nc.scalar.activation(warm, warm, EXP)
            nc.vector.memset(ones_pad[:, 0:64], 1.0)
            nc.vector.memset(ones_pad[:, 64:128], 0.0)

            # ---- input DMA (Q/K path first: it gates the pipeline) ----
            nc.sync.dma_start(out=wqk8, in_=wqk8_d[:, :, :])
            nc.sync.dma_start(out=xt8, in_=xt8_d[:, :, :])
            for g in range(2):
                nc.sync.dma_start(out=wo_t[g], in_=wo_d[g * P:(g + 1) * P, :])
            nc.sync.dma_start(out=ident, in_=id_d[:, :])
            nc.sync.dma_start(out=wvb, in_=wvb_d[:, :, :])
            nc.sync.dma_start(out=xtb, in_=xtb_d[:, :, :])

            PSLOT = [(U[0], 0), (U[0], 512), (U[1], 0), (U[1], 512),
                     (ps_c, 0), (ps_c, 512), (ps_c, 1024), (ps_c, 1536)]

            # wqk8 column layout: [Q01, Q23, K01, K23] x 128
            def qk_group(mcol, dst, nt, half, slot):
                """fp8 DoubleRow projection subgroup: 4 K=256 matmuls into a
                borrowed 256-col psum chunk, then f32->bf16 copy out."""
                tl, off = PSLOT[slot]
                pt = tl[:, off:off + 256]
                lo = nt * 512 + half * 256
                for j in range(4):
                    nc.tensor.matmul(
                        pt, wqk8[:, 2 * j:2 * j + 2, mcol * P:(mcol + 1) * P],
                        xt8[:, 2 * j:2 * j + 2, lo:lo + 256],
                        start=(j == 0), stop=(j == 3), perf_mode=DR)
                nc.vector.tensor_copy(out=dst[:, lo:lo + 256], in_=pt)

            def v_group(mc, nt, half, slot):
                """bf16 V projection subgroup: 8 K=128 matmuls, N=256."""
                tl, off = PSLOT[slot]
                pt = tl[:, off:off + 256]
                lo = nt * 512 + half * 256
                for kc in range(8):
                    nc.tensor.matmul(
                        pt, wvb[:, kc, mc * P:(mc + 1) * P],
                        xtb[:, kc, lo:lo + 256],
                        start=(kc == 0), stop=(kc == 7))
                nc.vector.tensor_copy(out=vt[mc][:, lo:lo + 256], in_=pt)

            def ph4p0(sc, nh, slot_c):
                """Pair-0 phase-4 partial chunk into a borrowed ps_c bank,
                copied to bf16 SBUF for the tail's identity re-injection."""
                pt = ps_c[:, slot_c * 512:(slot_c + 1) * 512]
                nc.tensor.matmul(
                    pt, vt[0][:, sc * P:(sc + 1) * P],
                    wo_t[0][:, nh * 512:(nh + 1) * 512], start=True, stop=True)
                nc.vector.tensor_copy(
                    out=out_sb[:, sc, nh * 512:(nh + 1) * 512], in_=pt)

            def emit_scores_half(g, qc, hh):
                """Head A and B score matmuls for k-half hh, interleaved so
                they run concurrently in PE row groups 0-1 / 2-3."""
                for n in range(2):
                    ks = hh * 1024 + n * 512
                    for h in range(2):
                        bp = h * 64
                        nc.tensor.matmul(
                            U[h][:, n * 512:(n + 1) * 512],
                            qt[g][bp:bp + 64, qc * P:(qc + 1) * P],
                            kt[g][bp:bp + 64, ks:ks + 512],
                            start=True, stop=True)

            def emit_exp_half(h, t, hh):
                nc.scalar.activation(
                    e_t[h][t % 2][:, hh * 1024:(hh + 1) * 1024], U[h],
                    EXP, scale=0.125,
                    accum_out=r_t[h][t % 2][:, hh:hh + 1])

            def emit_accum(t):
                """acc_h += E * (1/rowsum), all-bf16: head A fused on DVE;
                head B on GPSIMD (which lacks TensorScalarPtr) as a
                broadcast-multiply plus add."""
                qc = t % QC
                i = t % 2
                for h in range(2):
                    nc.vector.tensor_tensor(rs_t[h][i], r_t[h][i][:, 0:1],
                                            r_t[h][i][:, 1:2], ADD)
                    nc.vector.reciprocal(rs_t[h][i], rs_t[h][i])
                if qc == 0:
                    nc.vector.tensor_scalar(acc[0], e_t[0][i], rs_t[0][i],
                                            None, MULT)
                    nc.gpsimd.tensor_tensor(
                        acc[1], e_t[1][i],
                        rs_t[1][i].to_broadcast([P, S]), MULT)
                else:
                    nc.vector.scalar_tensor_tensor(acc[0], e_t[0][i],
                                                   rs_t[0][i], acc[0],
                                                   MULT, ADD)
                    nc.gpsimd.tensor_tensor(
                        tmp_b, e_t[1][i],
                        rs_t[1][i].to_broadcast([P, S]), MULT)
                    nc.gpsimd.tensor_tensor(acc[1], acc[1], tmp_b, ADD)

            def emit_colsum(g):
                """Partition-reduce acc into ps_c: head A via the wide
                [ones|zeros] weight (bank-clearing start), head B accumulated
                at base partition 64."""
                for c in range(4):
                    cs = slice(c * 512, (c + 1) * 512)
                    nc.tensor.matmul(ps_c[:, cs], ones_pad, acc[0][:, cs],
                                     start=True, stop=False,
                                     skip_group_check=True)
                    nc.tensor.matmul(ps_c[64:128, cs], ones_pad[:, 0:64],
                                     acc[1][:, cs], start=False, stop=True,
                                     skip_group_check=True)

            # ---- lead-in: K01/Q01 projection (gates first scores) ----
            for i, mcol in enumerate((2, 0)):
                dst = kt[0] if mcol == 2 else qt[0]
                for nt in range(4):
                    for half in range(2):
                        qk_group(mcol, dst, nt, half,
                                 slot=(i * 8 + nt * 2 + half) % 8)

            # Borrowed-window queue, ordered by deadline: V01 gates the
            # pair-0 v-scale (t=17); kt[1] and qt[1] nt0 gate pair-1 scores
            # (t=16); qt[1] ntX gates t=16+4X; V23 gates the pair-1 v-scale.
            # wqk8 mcols are [Q01, Q23, K01, K23]: Q23 = mcol 1, K23 = mcol 3.
            win = [(v_group, (0, nt, half)) for nt in range(4) for half in range(2)]
            win += [(qk_group, (1, qt[1], 0, half)) for half in range(2)]
            win += [(qk_group, (3, kt[1], nt, half))
                    for nt in range(4) for half in range(2)]
            win += [(qk_group, (1, qt[1], nt, half))
                    for nt in (1, 2, 3) for half in range(2)]
            win += [(v_group, (1, nt, half)) for nt in range(4) for half in range(2)]
            ph4q = [(sc, nh) for sc in range(QC) for nh in range(2)]

            # ---- main loop: flat chunk stream t = g*16 + qc ----
            for t in range(2 * QC):
                g, qc = divmod(t, QC)
                # half 0: scores A/B then exps
                emit_scores_half(g, qc, 0)
                emit_exp_half(0, t, 0)
                emit_exp_half(1, t, 0)
                if t == QC:
                    emit_colsum(0)
                if t == QC + 1:
                    # pair-0 colsums in ps_c: v *= colsum (before ph4p0
                    # starts borrowing ps_c banks below)
                    nc.vector.tensor_tensor(vt[0], vt[0], ps_c, MULT)
                # half 1
                emit_scores_half(g, qc, 1)
                emit_exp_half(0, t, 1)
                emit_exp_half(1, t, 1)
                emit_accum(t)
                # borrowed-window projection work
                if t >= 1 and win:
                    fn, args = win.pop(0)
                    fn(*args, slot=2 + (t % 2))      # U1 halves
                if t >= 13 and win:
                    fn, args = win.pop(0)
                    fn(*args, slot=t % 2)            # U0 halves
                # pair-0 phase-4 partials into idle ps_c banks
                if t >= QC + 2:
                    for k in range(3):
                        if ph4q:
                            sc, nh = ph4q.pop(0)
                            ph4p0(sc, nh, slot_c=(sc * 2 + nh) % 4)

            # ---- tail ----
            emit_colsum(1)
            nc.vector.tensor_tensor(vt[1], vt[1], ps_c, MULT)
            for sc, nh in ph4q:
                ph4p0(sc, nh, slot_c=(sc * 2 + nh) % 4)
            for sc in range(QC):
                st = stage[sc % 2]
                for nh in range(2):
                    tl, off = PSLOT[(sc * 2 + nh) % 8]
                    pt = tl[:, off:off + 512]
                    nc.tensor.matmul(
                        pt, vt[1][:, sc * P:(sc + 1) * P],
                        wo_t[1][:, nh * 512:(nh + 1) * 512],
                        start=True, stop=False)
                    nc.tensor.matmul(
                        pt, ident, out_sb[:, sc, nh * 512:(nh + 1) * 512],
                        start=False, stop=True)
                    dst = st[:, nh * 512:(nh + 1) * 512]
                    if nh == 0:
                        nc.vector.tensor_copy(out=dst, in_=pt)
                    else:
                        nc.scalar.copy(out=dst, in_=pt)
                nc.sync.dma_start(out=out_d[sc * P:(sc + 1) * P, :], in_=st)

    nc.compile()
    _CACHE["nc"] = nc
    return nc


def kernel(x: np.ndarray, w_qkv: np.ndarray, w_o: np.ndarray) -> np.ndarray:
    import ml_dtypes
    from concourse.bass_utils import run_bass_kernel_spmd

    nc = _build()

    def ilv(a, dt):
        # [1024, n] -> [128, 8, n]: hidden index (kc*128 + p) -> (p, kc)
        return np.ascontiguousarray(
            a.reshape(8, P, -1).transpose(1, 0, 2)).astype(dt)

    f8 = ml_dtypes.float8_e4m3fn
    bf = ml_dtypes.bfloat16
    xT = [np.ascontiguousarray(x[b].T) for b in range(B)]
    xt8 = [ilv(t, f8) for t in xT]
    xtb = [ilv(t, bf) for t in xT]
    identity = np.eye(P, dtype=bf)

    in_maps = []
    for c in range(N_CORES):
        b, g = divmod(c, 4)
        qcols = w_qkv[:, 256 * g:256 * g + 256]
        kcols = w_qkv[:, NH * HD + 256 * g:NH * HD + 256 * g + 256]
        vcols = w_qkv[:, 2 * NH * HD + 256 * g:2 * NH * HD + 256 * g + 256]
        wqk = np.concatenate([qcols, kcols], axis=1)  # [1024, 512]
        in_maps.append({
            "xt8": xt8[b],
            "xtb": xtb[b],
            "wqk8": ilv(wqk, f8),
            "wvb": ilv(vcols, bf),
            "wo": np.ascontiguousarray(w_o[256 * g:256 * g + 256, :]).astype(bf),
            "ident": identity,
        })

    res = run_bass_kernel_spmd(nc, in_maps, list(range(N_CORES)),
                               **_CACHE.get("run_kwargs", {}))
    _CACHE["last_result"] = res

    out = np.zeros((B, S, HIDDEN), np.float32)
    for c in range(N_CORES):
        out[c // 4] += np.asarray(res.results[c]["out"]).astype(np.float32)
    return out


# revision 17
# speedup vs baseline: 1.3900x; 1.3900x over previous
"""Trainium2 Bass kernel for nn_BaselineAttention_36172214567310.

Reference computation (note the einsum 'bhqk,bhkd->bhkd' bug: the "attention
output" is v scaled by the column-sums of the softmax matrix):

    qkv = x @ w_qkv                       # [b, s, 3*H*D]
    q, k, v = split(qkv)                  # per head
    P = softmax(q @ k^T / sqrt(D))        # [q, k] rows sum to 1
    colsum[k] = sum_q P[q, k]
    values[k, :] = v[k, :] * colsum_h[k]
    out = values @ w_o

Sharding: 8 cores = 2 batches x 4 head-groups (4 heads each).

Per-core pipeline. The ACT exp stream is the pacing engine (~180us:
128 x [128,1024] ACTIVATE + fused-rowsum drain); everything else is
overlapped under it:
  - Q/K projection in fp8 DoubleRow (K=256 per matmul), V in bf16.
  - Scores bf16, K=64, head pair A/B row-tiled (lhsT base partitions 0/64)
    so the two heads' score matmuls run concurrently; A fills U0, B fills
    U1, in [128,1024] halves ping-ponged against the exps.
  - exp on ACT -> bf16 E tiles in SBUF + accum_out rowsum per half.
  - colsum matvec per chunk: bf16, weights 1/rowsum replicated over 64
    stationary columns, split into four 64x64 PE quadrants
    (A-low/B-high then A-high/B-low) so the two matmuls of each wave run
    concurrently and their weight loads pull ahead; accumulated into a
    RESIDENT 4-bank PSUM tile ps_c across all 16 chunks (the first matmul
    of a pair uses a zero-padded [64,128] weight so its bank-wide
    has_written clear covers head B's partitions).
  - v *= colsum straight from ps_c, then a tail computes
    out = values @ w_o with both head-pairs accumulated in PSUM, copies
    alternating DVE/ACT, bf16 chunk-streamed DMA out. Host sums the 4
    group partials per batch in f32.

PSUM: U0, U1 ([128,1024] f32, 2 banks each) + ps_c ([128,2048] f32,
4 banks) = all 8 banks. Projection leftovers borrow idle U windows.
"""

import sys

sys.path.insert(0, "/opt/trn_rl_repo")

import numpy as np

B, S, HIDDEN = 2, 2048, 1024
NH, HD = 16, 64
N_CORES = 8
P = 128
QC = S // P          # 16 q chunks of 128

_CACHE = {}


def _build():
    if "nc" in _CACHE:
        return _CACHE["nc"]

    import concourse.mybir as mybir
    import concourse.tile as tile
    from concourse import bacc

    F32 = mybir.dt.float32
    BF16 = mybir.dt.bfloat16
    F8E4 = mybir.dt.float8e4
    EXP = mybir.ActivationFunctionType.Exp
    ADD = mybir.AluOpType.add
    MULT = mybir.AluOpType.mult
    DR = mybir.MatmulPerfMode.DoubleRow

    nc = bacc.Bacc()
    xt8_d = nc.declare_dram_parameter("xt8", [P, 8, S], F8E4, isOutput=False)
    xtb_d = nc.declare_dram_parameter("xtb", [P, 8, S], BF16, isOutput=False)
    wqk8_d = nc.declare_dram_parameter("wqk8", [P, 8, 512], F8E4, isOutput=False)
    wvb_d = nc.declare_dram_parameter("wvb", [P, 8, 256], BF16, isOutput=False)
    wo_d = nc.declare_dram_parameter("wo", [256, HIDDEN], BF16, isOutput=False)
    out_d = nc.declare_dram_parameter("out", [S, HIDDEN], BF16, isOutput=True)

    with tile.TileContext(nc) as tc:
        with tc.tile_pool(name="sb", bufs=1) as sb, \
             tc.tile_pool(name="ps", bufs=1, space="PSUM") as ps:

            # ---- persistent SBUF tiles ----
            qt = [sb.tile([P, S], BF16, name=f"qt{g}") for g in range(2)]
            kt = [sb.tile([P, S], BF16, name=f"kt{g}") for g in range(2)]
            vt = [sb.tile([P, S], BF16, name=f"vt{g}") for g in range(2)]
            xt8 = sb.tile([P, 8, S], F8E4, name="xt8")
            xtb = sb.tile([P, 8, S], BF16, name="xtb")
            wqk8 = sb.tile([P, 8, 512], F8E4, name="wqk8")
            wvb = sb.tile([P, 8, 256], BF16, name="wvb")
            wo_t = [sb.tile([P, HIDDEN], BF16, name=f"wo{g}") for g in range(2)]
            # E + rowsums, ping-pong per head over chunk parity
            e_t = [[sb.tile([P, S], BF16, name=f"e{h}{i}") for i in range(2)]
                   for h in range(2)]
            r_t = [[sb.tile([P, 2], F32, name=f"r{h}{i}") for i in range(2)]
                   for h in range(2)]
            rs_t = [[sb.tile([P, 1], F32, name=f"rs{h}{i}") for i in range(2)]
                    for h in range(2)]
            wr_t = [[sb.tile([P, 64], BF16, name=f"wr{h}{i}") for i in range(2)]
                    for h in range(2)]
            # zero-padded first-matvec weights (A-low cols 0-63, zeros after)
            wr_pad = sb.tile([P, P], BF16, name="wr_pad")
            stage = [sb.tile([P, HIDDEN], BF16, name=f"stage{i}") for i in range(2)]
            warm = sb.tile([P, 1], F32, name="warm")

            # PSUM: 2+2 banks of score halves, 4 banks colsum accumulator
            U = [ps.tile([P, 1024], F32, name=f"U{i}") for i in range(2)]
            ps_c = ps.tile([P, S], F32, name="ps_c")

            # ---- constants + exp table preload (overlap input DMA) ----
            nc.vector.memset(warm, 0.0)
            nc.scalar.activation(warm, warm, EXP)
            nc.vector.memset(wr_pad[:, 64:128], 0.0)

            # ---- input DMA (Q/K path first: it gates the pipeline) ----
            nc.sync.dma_start(out=wqk8, in_=wqk8_d[:, :, :])
            nc.sync.dma_start(out=xt8, in_=xt8_d[:, :, :])
            for g in range(2):
                nc.sync.dma_start(out=wo_t[g], in_=wo_d[g * P:(g + 1) * P, :])
            nc.sync.dma_start(out=wvb, in_=wvb_d[:, :, :])
            nc.sync.dma_start(out=xtb, in_=xtb_d[:, :, :])

            PSLOT = [(U[0], 0), (U[0], 512), (U[1], 0), (U[1], 512),
                     (ps_c, 0), (ps_c, 512), (ps_c, 1024), (ps_c, 1536)]

            # wqk8 column layout: [Q01, Q23, K01, K23] x 128
            def qk_group(mcol, dst, nt, half, slot):
                """fp8 DoubleRow projection subgroup: 4 K=256 matmuls into a
                borrowed 256-col psum chunk, then f32->bf16 copy out."""
                tl, off = PSLOT[slot]
                pt = tl[:, off:off + 256]
                lo = nt * 512 + half * 256
                for j in range(4):
                    nc.tensor.matmul(
                        pt, wqk8[:, 2 * j:2 * j + 2, mcol * P:(mcol + 1) * P],
                        xt8[:, 2 * j:2 * j + 2, lo:lo + 256],
                        start=(j == 0), stop=(j == 3), perf_mode=DR)
                nc.vector.tensor_copy(out=dst[:, lo:lo + 256], in_=pt)

            def v_group(mc, nt, half, slot):
                """bf16 V projection subgroup: 8 K=128 matmuls, N=256."""
                tl, off = PSLOT[slot]
                pt = tl[:, off:off + 256]
                lo = nt * 512 + half * 256
                for kc in range(8):
                    nc.tensor.matmul(
                        pt, wvb[:, kc, mc * P:(mc + 1) * P],
                        xtb[:, kc, lo:lo + 256],
                        start=(kc == 0), stop=(kc == 7))
                nc.vector.tensor_copy(out=vt[mc][:, lo:lo + 256], in_=pt)

            def emit_scores_half(g, qc, hh):
                """Head A and B score matmuls for k-half hh, interleaved so
                they run concurrently in PE row groups 0-1 / 2-3."""
                for n in range(2):
                    ks = hh * 1024 + n * 512
                    for h in range(2):
                        bp = h * 64
                        nc.tensor.matmul(
                            U[h][:, n * 512:(n + 1) * 512],
                            qt[g][bp:bp + 64, qc * P:(qc + 1) * P],
                            kt[g][bp:bp + 64, ks:ks + 512],
                            start=True, stop=True)

            def emit_exp_half(h, t, hh):
                nc.scalar.activation(
                    e_t[h][t % 2][:, hh * 1024:(hh + 1) * 1024], U[h],
                    EXP, scale=0.125,
                    accum_out=r_t[h][t % 2][:, hh:hh + 1])

            def emit_wr(h, t):
                """1/rowsum matvec weights for chunk t of head h (bf16,
                replicated across 64 stationary cols; chunk 0 of a pair also
                fills the zero-padded wide weight for the bank clear)."""
                i = t % 2
                nc.vector.tensor_tensor(rs_t[h][i], r_t[h][i][:, 0:1],
                                        r_t[h][i][:, 1:2], ADD)
                nc.vector.reciprocal(rs_t[h][i], rs_t[h][i])
                nc.vector.tensor_copy(
                    out=wr_t[h][i], in_=rs_t[h][i].to_broadcast([P, 64]))
                if h == 0 and t % QC == 0:
                    nc.vector.tensor_copy(
                        out=wr_pad[:, 0:64],
                        in_=rs_t[h][i].to_broadcast([P, 64]))

            def emit_mv(t):
                """Colsum matvec for chunk t into the resident ps_c: bf16
                K=128 matmuls, head A -> psum partitions 0-63 and head B ->
                64-127 (col groups 0-1 / 2-3), accumulated across the pair's
                16 chunks. The pair's first matmul uses the zero-padded wide
                weight so its bank-wide has_written clear covers head B."""
                qc = t % QC
                i = t % 2
                last = (qc == QC - 1)
                for c in range(4):
                    cs = slice(c * 512, (c + 1) * 512)
                    if qc == 0:
                        nc.tensor.matmul(
                            ps_c[:, cs], wr_pad, e_t[0][i][:, cs],
                            start=True, stop=False, skip_group_check=True)
                    else:
                        nc.tensor.matmul(
                            ps_c[0:64, cs], wr_t[0][i], e_t[0][i][:, cs],
                            start=False, stop=last, skip_group_check=True)
                    nc.tensor.matmul(
                        ps_c[64:128, cs], wr_t[1][i], e_t[1][i][:, cs],
                        start=False, stop=last, skip_group_check=True)

            # ---- lead-in: K01/Q01 projection (gates first scores) ----
            for i, mcol in enumerate((2, 0)):
                dst = kt[0] if mcol == 2 else qt[0]
                for nt in range(4):
                    for half in range(2):
                        qk_group(mcol, dst, nt, half,
                                 slot=(i * 8 + nt * 2 + half) % 8)

            # Borrowed-window queue, ordered by deadline: V01 gates the
            # pair-0 v-scale (t=17); kt[1] and qt[1] nt0 gate pair-1 scores
            # (t=16); qt[1] ntX gates t=16+4X; V23 gates the pair-1 v-scale.
            # wqk8 mcols are [Q01, Q23, K01, K23]: Q23 = mcol 1, K23 = mcol 3.
            win = [(v_group, (0, nt, half)) for nt in range(4) for half in range(2)]
            win += [(qk_group, (1, qt[1], 0, half)) for half in range(2)]
            win += [(qk_group, (3, kt[1], nt, half))
                    for nt in range(4) for half in range(2)]
            win += [(qk_group, (1, qt[1], nt, half))
                    for nt in (1, 2, 3) for half in range(2)]
            win += [(v_group, (1, nt, half)) for nt in range(4) for half in range(2)]

            # ---- main loop: flat chunk stream t = g*16 + qc ----
            for t in range(2 * QC):
                g, qc = divmod(t, QC)
                # half 0: scores A/B then exps
                emit_scores_half(g, qc, 0)
                emit_exp_half(0, t, 0)
                emit_exp_half(1, t, 0)
                if t == QC + 1:
                    # pair-0 colsums complete: v *= colsum (before pair-1's
                    # first matvec below clears ps_c)
                    nc.vector.tensor_tensor(vt[0], vt[0], ps_c, MULT)
                # matvec for the previous chunk (weights built last iter)
                if t >= 1:
                    emit_mv(t - 1)
                # half 1
                emit_scores_half(g, qc, 1)
                emit_exp_half(0, t, 1)
                emit_exp_half(1, t, 1)
                for h in range(2):
                    emit_wr(h, t)
                # borrowed-window projection work
                if t >= 1 and win:
                    fn, args = win.pop(0)
                    fn(*args, slot=2 + (t % 2))      # U1 halves
                if t >= 13 and win:
                    fn, args = win.pop(0)
                    fn(*args, slot=t % 2)            # U0 halves

            # ---- tail ----
            emit_mv(2 * QC - 1)
            nc.vector.tensor_tensor(vt[1], vt[1], ps_c, MULT)
            for sc in range(QC):
                st = stage[sc % 2]
                for nh in range(2):
                    tl, off = PSLOT[(sc * 2 + nh) % 8]
                    pt = tl[:, off:off + 512]
                    for g in range(2):
                        nc.tensor.matmul(
                            pt, vt[g][:, sc * P:(sc + 1) * P],
                            wo_t[g][:, nh * 512:(nh + 1) * 512],
                            start=(g == 0), stop=(g == 1))
                    dst = st[:, nh * 512:(nh + 1) * 512]
                    if nh == 0:
                        nc.vector.tensor_copy(out=dst, in_=pt)
                    else:
                        nc.scalar.copy(out=dst, in_=pt)
                nc.sync.dma_start(out=out_d[sc * P:(sc + 1) * P, :], in_=st)

    nc.compile()
    _CACHE["nc"] = nc
    return nc


def kernel(x: np.ndarray, w_qkv: np.ndarray, w_o: np.ndarray) -> np.ndarray:
    import ml_dtypes
    from concourse.bass_utils import run_bass_kernel_spmd

    nc = _build()

    def ilv(a, dt):
        # [1024, n] -> [128, 8, n]: hidden index (kc*128 + p) -> (p, kc)
        return np.ascontiguousarray(
            a.reshape(8, P, -1).transpose(1, 0, 2)).astype(dt)

    f8 = ml_dtypes.float8_e4m3fn
    bf = ml_dtypes.bfloat16
    xT = [np.ascontiguousarray(x[b].T) for b in range(B)]
    xt8 = [ilv(t, f8) for t in xT]
    xtb = [ilv(t, bf) for t in xT]

    in_maps = []
    for c in range(N_CORES):
        b, g = divmod(c, 4)
        qcols = w_qkv[:, 256 * g:256 * g + 256]
        kcols = w_qkv[:, NH * HD + 256 * g:NH * HD + 256 * g + 256]
        vcols = w_qkv[:, 2 * NH * HD + 256 * g:2 * NH * HD + 256 * g + 256]
        wqk = np.concatenate([qcols, kcols], axis=1)  # [1024, 512]
        in_maps.append({
            "xt8": xt8[b],
            "xtb": xtb[b],
            "wqk8": ilv(wqk, f8),
            "wvb": ilv(vcols, bf),
            "wo": np.ascontiguousarray(w_o[256 * g:256 * g + 256, :]).astype(bf),
        })

    res = run_bass_kernel_spmd(nc, in_maps, list(range(N_CORES)),
                               **_CACHE.get("run_kwargs", {}))
    _CACHE["last_result"] = res

    out = np.zeros((B, S, HIDDEN), np.float32)
    for c in range(N_CORES):
        out[c // 4] += np.asarray(res.results[c]["out"]).astype(np.float32)
    return out


# revision 22
# speedup vs baseline: 1.5309x; 1.1013x over previous
"""Trainium2 Bass kernel for nn_BaselineAttention_36172214567310.

Reference computation (note the einsum 'bhqk,bhkd->bhkd' bug: the "attention
output" is v scaled by the column-sums of the softmax matrix):

    qkv = x @ w_qkv                       # [b, s, 3*H*D]
    q, k, v = split(qkv)                  # per head
    P = softmax(q @ k^T / sqrt(D))        # [q, k] rows sum to 1
    colsum[k] = sum_q P[q, k]
    values[k, :] = v[k, :] * colsum_h[k]
    out = values @ w_o

Sharding: 8 cores = 2 batches x 4 head-groups (4 heads each).

Per-core pipeline. The ACT exp stream is the pacing engine (~180us:
128 x [128,1024] ACTIVATE + fused-rowsum drain); everything else is
overlapped under it:
  - Q/K projection in fp8 DoubleRow (K=256 per matmul), V in bf16.
  - Scores bf16, K=64, head pair A/B row-tiled (lhsT base partitions 0/64)
    so the two heads' score matmuls run concurrently; A fills U0, B fills
    U1, in [128,1024] halves ping-ponged against the exps.
  - exp on ACT -> bf16 E tiles in SBUF + accum_out rowsum per half.
  - colsum matvec per chunk: bf16, weights 1/rowsum replicated over 64
    stationary columns, split into four 64x64 PE quadrants
    (A-low/B-high then A-high/B-low) so the two matmuls of each wave run
    concurrently and their weight loads pull ahead; accumulated into a
    RESIDENT 4-bank PSUM tile ps_c across all 16 chunks (the first matmul
    of a pair uses a zero-padded [64,128] weight so its bank-wide
    has_written clear covers head B's partitions).
  - v *= colsum straight from ps_c, then a tail computes
    out = values @ w_o with both head-pairs accumulated in PSUM, copies
    alternating DVE/ACT, bf16 chunk-streamed DMA out. Host sums the 4
    group partials per batch in f32.

PSUM: U0, U1 ([128,1024] f32, 2 banks each) + ps_c ([128,2048] f32,
4 banks) = all 8 banks. Projection leftovers borrow idle U windows.
"""

import sys

sys.path.insert(0, "/opt/trn_rl_repo")

import numpy as np

B, S, HIDDEN = 2, 2048, 1024
NH, HD = 16, 64
N_CORES = 8
P = 128
QC = S // P          # 16 q chunks of 128

_CACHE = {}


def _build():
    if "nc" in _CACHE:
        return _CACHE["nc"]

    import concourse.mybir as mybir
    import concourse.tile as tile
    from concourse import bacc

    F32 = mybir.dt.float32
    BF16 = mybir.dt.bfloat16
    F8E4 = mybir.dt.float8e4
    EXP = mybir.ActivationFunctionType.Exp
    ADD = mybir.AluOpType.add
    MULT = mybir.AluOpType.mult
    DR = mybir.MatmulPerfMode.DoubleRow

    nc = bacc.Bacc()
    xt8_d = nc.declare_dram_parameter("xt8", [P, 8, S], F8E4, isOutput=False)
    xtb_d = nc.declare_dram_parameter("xtb", [P, 8, S], BF16, isOutput=False)
    wqk8_d = nc.declare_dram_parameter("wqk8", [P, 8, 512], F8E4, isOutput=False)
    wvb_d = nc.declare_dram_parameter("wvb", [P, 8, 256], BF16, isOutput=False)
    wo_d = nc.declare_dram_parameter("wo", [256, HIDDEN], BF16, isOutput=False)
    out_d = nc.declare_dram_parameter("out", [S, HIDDEN], BF16, isOutput=True)

    with tile.TileContext(nc) as tc:
        with tc.tile_pool(name="sb", bufs=1) as sb, \
             tc.tile_pool(name="ps", bufs=1, space="PSUM") as ps:

            # ---- persistent SBUF tiles ----
            qt = [sb.tile([P, S], BF16, name=f"qt{g}") for g in range(2)]
            kt = [sb.tile([P, S], BF16, name=f"kt{g}") for g in range(2)]
            vt = [sb.tile([P, S], BF16, name=f"vt{g}") for g in range(2)]
            xt8 = sb.tile([P, 8, S], F8E4, name="xt8")
            xtb = sb.tile([P, 8, S], BF16, name="xtb")
            wqk8 = sb.tile([P, 8, 512], F8E4, name="wqk8")
            wvb = sb.tile([P, 8, 256], BF16, name="wvb")
            wo_t = [sb.tile([P, HIDDEN], BF16, name=f"wo{g}") for g in range(2)]
            # E + rowsums, ping-pong per head over chunk parity
            e_t = [[sb.tile([P, S], BF16, name=f"e{h}{i}") for i in range(2)]
                   for h in range(2)]
            r_t = [[sb.tile([P, 2], F32, name=f"r{h}{i}") for i in range(2)]
                   for h in range(2)]
            rs_t = [[sb.tile([P, 1], F32, name=f"rs{h}{i}") for i in range(2)]
                    for h in range(2)]
            wr_t = [[sb.tile([P, 64], BF16, name=f"wr{h}{i}") for i in range(2)]
                    for h in range(2)]
            # zero-padded first-matvec weights (A-low cols 0-63, zeros after)
            wr_pad = sb.tile([P, P], BF16, name="wr_pad")
            # all-zero 1-col weights for HAM-warming filler matmuls
            zeros_w = sb.tile([P, 1], BF16, name="zeros_w")
            stage = [sb.tile([P, HIDDEN], BF16, name=f"stage{i}") for i in range(2)]
            warm = sb.tile([P, 1], F32, name="warm")

            # PSUM: 2+2 banks of score halves, 4 banks colsum accumulator
            U = [ps.tile([P, 1024], F32, name=f"U{i}") for i in range(2)]
            ps_c = ps.tile([P, S], F32, name="ps_c")

            # ---- constants + exp table preload (overlap input DMA) ----
            nc.vector.memset(warm, 0.0)
            nc.scalar.activation(warm, warm, EXP)
            nc.vector.memset(wr_pad[:, 64:128], 0.0)
            nc.vector.memset(zeros_w, 0.0)

            # ---- input DMA (Q/K path first: it gates the pipeline) ----
            nc.sync.dma_start(out=wqk8, in_=wqk8_d[:, :, :])
            nc.sync.dma_start(out=xt8, in_=xt8_d[:, :, :])
            nc.sync.dma_start(out=wvb, in_=wvb_d[:, :, :])
            nc.sync.dma_start(out=xtb, in_=xtb_d[:, :, :])
            for g in range(2):
                nc.sync.dma_start(out=wo_t[g], in_=wo_d[g * P:(g + 1) * P, :])

            PSLOT = [(U[0], 0), (U[0], 512), (U[1], 0), (U[1], 512),
                     (ps_c, 0), (ps_c, 512), (ps_c, 1024), (ps_c, 1536)]

            # wqk8 column layout: [Q01, Q23, K01, K23] x 128
            def qk_group(mcol, dst, nt, half, slot):
                """fp8 DoubleRow projection subgroup: 4 K=256 matmuls into a
                borrowed 256-col psum chunk, then f32->bf16 copy out."""
                tl, off = PSLOT[slot]
                pt = tl[:, off:off + 256]
                lo = nt * 512 + half * 256
                for j in range(4):
                    nc.tensor.matmul(
                        pt, wqk8[:, 2 * j:2 * j + 2, mcol * P:(mcol + 1) * P],
                        xt8[:, 2 * j:2 * j + 2, lo:lo + 256],
                        start=(j == 0), stop=(j == 3), perf_mode=DR)
                nc.vector.tensor_copy(out=dst[:, lo:lo + 256], in_=pt)

            def v_group(mc, nt, half, slot):
                """bf16 V projection subgroup: 8 K=128 matmuls, N=256."""
                tl, off = PSLOT[slot]
                pt = tl[:, off:off + 256]
                lo = nt * 512 + half * 256
                for kc in range(8):
                    nc.tensor.matmul(
                        pt, wvb[:, kc, mc * P:(mc + 1) * P],
                        xtb[:, kc, lo:lo + 256],
                        start=(kc == 0), stop=(kc == 7))
                nc.vector.tensor_copy(out=vt[mc][:, lo:lo + 256], in_=pt)

            def emit_scores_half(g, qc, hh):
                """Head A and B score matmuls for k-half hh, interleaved so
                they run concurrently in PE row groups 0-1 / 2-3."""
                for n in range(2):
                    ks = hh * 1024 + n * 512
                    for h in range(2):
                        bp = h * 64
                        nc.tensor.matmul(
                            U[h][:, n * 512:(n + 1) * 512],
                            qt[g][bp:bp + 64, qc * P:(qc + 1) * P],
                            kt[g][bp:bp + 64, ks:ks + 512],
                            start=True, stop=True)

            def emit_exp_half(h, t, hh):
                nc.scalar.activation(
                    e_t[h][t % 2][:, hh * 1024:(hh + 1) * 1024], U[h],
                    EXP, scale=0.125,
                    accum_out=r_t[h][t % 2][:, hh:hh + 1])

            def emit_wr(h, t):
                """1/rowsum matvec weights for chunk t of head h (bf16,
                replicated across 64 stationary cols; chunk 0 of a pair also
                fills the zero-padded wide weight for the bank clear)."""
                i = t % 2
                nc.vector.tensor_tensor(rs_t[h][i], r_t[h][i][:, 0:1],
                                        r_t[h][i][:, 1:2], ADD)
                nc.vector.reciprocal(rs_t[h][i], rs_t[h][i])
                nc.vector.tensor_copy(
                    out=wr_t[h][i], in_=rs_t[h][i].to_broadcast([P, 64]))
                if h == 0 and t % QC == 0:
                    nc.vector.tensor_copy(
                        out=wr_pad[:, 0:64],
                        in_=rs_t[h][i].to_broadcast([P, 64]))

            def emit_mv(t):
                """Colsum matvec for chunk t into the resident ps_c: bf16
                K=128 matmuls, head A -> psum partitions 0-63 and head B ->
                64-127 (col groups 0-1 / 2-3), accumulated across the pair's
                16 chunks. The pair's first matmul uses the zero-padded wide
                weight so its bank-wide has_written clear covers head B."""
                qc = t % QC
                i = t % 2
                last = (qc == QC - 1)
                for c in range(4):
                    cs = slice(c * 512, (c + 1) * 512)
                    if qc == 0:
                        nc.tensor.matmul(
                            ps_c[:, cs], wr_pad, e_t[0][i][:, cs],
                            start=True, stop=False, skip_group_check=True)
                    else:
                        nc.tensor.matmul(
                            ps_c[0:64, cs], wr_t[0][i], e_t[0][i][:, cs],
                            start=False, stop=last, skip_group_check=True)
                    nc.tensor.matmul(
                        ps_c[64:128, cs], wr_t[1][i], e_t[1][i][:, cs],
                        start=False, stop=last, skip_group_check=True)

            # ---- lead-in: K01/Q01 projection (gates first scores) ----
            for i, mcol in enumerate((2, 0)):
                dst = kt[0] if mcol == 2 else qt[0]
                for nt in range(4):
                    for half in range(2):
                        qk_group(mcol, dst, nt, half,
                                 slot=(i * 8 + nt * 2 + half) % 8)

            # Borrowed-window queue, ordered by deadline AND DMA arrival
            # (the V groups need the late xtb DMA, so QK work goes first to
            # avoid head-of-line blocking the PE queue): qt[1] nt0 and kt[1]
            # gate pair-1 scores (t=16); V01 gates the pair-0 v-scale (t=17);
            # qt[1] ntX gates t=16+4X; V23 gates the pair-1 v-scale.
            # wqk8 mcols are [Q01, Q23, K01, K23]: Q23 = mcol 1, K23 = mcol 3.
            win = [(qk_group, (1, qt[1], 0, half)) for half in range(2)]
            win += [(qk_group, (3, kt[1], nt, half))
                    for nt in range(4) for half in range(2)]
            win += [(v_group, (0, nt, half)) for nt in range(4) for half in range(2)]
            win += [(qk_group, (1, qt[1], nt, half))
                    for nt in (1, 2, 3) for half in range(2)]
            win += [(v_group, (1, nt, half)) for nt in range(4) for half in range(2)]

            def emit_dummy(t, j):
                """Zero-weight matmul accumulating 0 into one ps_c partition:
                numerically a no-op, but keeps the PE activity contiguous so
                the HAM clock gate stays at full speed. Reads the previous
                chunk's E (no new dependencies)."""
                c = (t * 3 + j) % 4
                nc.tensor.matmul(
                    ps_c[0:1, c * 512:(c + 1) * 512], zeros_w,
                    e_t[0][(t - 1) % 2][:, c * 512:(c + 1) * 512],
                    start=False, stop=False, skip_group_check=True)

            # ---- main loop: flat chunk stream t = g*16 + qc ----
            for t in range(2 * QC):
                g, qc = divmod(t, QC)
                # half 0: scores A/B then exps
                emit_scores_half(g, qc, 0)
                if t >= 1:
                    emit_dummy(t, 0)
                emit_exp_half(0, t, 0)
                emit_exp_half(1, t, 0)
                if t == QC + 1:
                    # pair-0 colsums complete: v *= colsum (before pair-1's
                    # first matvec below clears ps_c)
                    nc.vector.tensor_tensor(vt[0], vt[0], ps_c, MULT)
                # matvec for the previous chunk (weights built last iter)
                if t >= 1:
                    emit_mv(t - 1)
                    emit_dummy(t, 1)
                # half 1
                emit_scores_half(g, qc, 1)
                if t >= 1:
                    emit_dummy(t, 2)
                emit_exp_half(0, t, 1)
                emit_exp_half(1, t, 1)
                for h in range(2):
                    emit_wr(h, t)
                # borrowed-window projection work
                if t >= 1 and win:
                    fn, args = win.pop(0)
                    fn(*args, slot=2 + (t % 2))      # U1 halves
                if t >= 13 and win:
                    fn, args = win.pop(0)
                    fn(*args, slot=t % 2)            # U0 halves

            # ---- tail ----
            emit_mv(2 * QC - 1)
            nc.vector.tensor_tensor(vt[1], vt[1], ps_c, MULT)
            for sc in range(QC):
                st = stage[sc % 2]
                for nh in range(2):
                    tl, off = PSLOT[(sc * 2 + nh) % 8]
                    pt = tl[:, off:off + 512]
                    for g in range(2):
                        nc.tensor.matmul(
                            pt, vt[g][:, sc * P:(sc + 1) * P],
                            wo_t[g][:, nh * 512:(nh + 1) * 512],
                            start=(g == 0), stop=(g == 1))
                    dst = st[:, nh * 512:(nh + 1) * 512]
                    if nh == 0:
                        nc.vector.tensor_copy(out=dst, in_=pt)
                    else:
                        nc.scalar.copy(out=dst, in_=pt)
                nc.sync.dma_start(out=out_d[sc * P:(sc + 1) * P, :], in_=st)

    nc.compile()
    _CACHE["nc"] = nc
    return nc


def kernel(x: np.ndarray, w_qkv: np.ndarray, w_o: np.ndarray) -> np.ndarray:
    import ml_dtypes
    from concourse.bass_utils import run_bass_kernel_spmd

    nc = _build()

    def ilv(a, dt):
        # [1024, n] -> [128, 8, n]: hidden index (kc*128 + p) -> (p, kc)
        return np.ascontiguousarray(
            a.reshape(8, P, -1).transpose(1, 0, 2)).astype(dt)

    f8 = ml_dtypes.float8_e4m3fn
    bf = ml_dtypes.bfloat16
    xT = [np.ascontiguousarray(x[b].T) for b in range(B)]
    xt8 = [ilv(t, f8) for t in xT]
    xtb = [ilv(t, bf) for t in xT]

    in_maps = []
    for c in range(N_CORES):
        b, g = divmod(c, 4)
        qcols = w_qkv[:, 256 * g:256 * g + 256]
        kcols = w_qkv[:, NH * HD + 256 * g:NH * HD + 256 * g + 256]
        vcols = w_qkv[:, 2 * NH * HD + 256 * g:2 * NH * HD + 256 * g + 256]
        wqk = np.concatenate([qcols, kcols], axis=1)  # [1024, 512]
        in_maps.append({
            "xt8": xt8[b],
            "xtb": xtb[b],
            "wqk8": ilv(wqk, f8),
            "wvb": ilv(vcols, bf),
            "wo": np.ascontiguousarray(w_o[256 * g:256 * g + 256, :]).astype(bf),
        })

    res = run_bass_kernel_spmd(nc, in_maps, list(range(N_CORES)),
                               **_CACHE.get("run_kwargs", {}))
    _CACHE["last_result"] = res

    out = np.zeros((B, S, HIDDEN), np.float32)
    for c in range(N_CORES):
        out[c // 4] += np.asarray(res.results[c]["out"]).astype(np.float32)
    return out
